# revision 1
# baseline (speedup 1.0000x reference)
"""Trainium2 Bass kernel for EnergyIrrepModulation.

Computes out[m, e, d] = x[m, d] * gates_full[e, d] where
gates = MLP(e_feat) : [nE, n_copies], expanded to [nE, D] via the static
irrep index map for IRREPS = [(64, 1), (32, 3), (16, 5)].

Sharding: data-parallel over M (4096 rows -> 512 rows per core, 8 cores).
Gates/MLP params are replicated; each core redundantly computes the tiny MLP.

Per-core device plan:
  1. All MLP params arrive packed in ONE [128, 1080] tensor (split into two
     DMAs so the first layer's weights land first); the host pre-transposes
     e_feat so no on-device transposes are needed.
  2. Tiny MLP on the tensor engine; biases+ReLU fused on the scalar engine
     (b3 is added along the free dim with a ones[100,1] @ b3[1,112] matmul).
  3. Gates broadcast to all 128 partitions: chunk 0 (e 0:25) via flatten-to-
     partition-0 + GPSIMD partition_broadcast (lowest latency, completes
     before the first multiply so it never contends with the vector engine);
     chunks 1-3 via a DRAM bounce + stride-0 source-read DMAs, deferred
     behind the chunk-0 broadcast so the 16 SDMA engines stay free for it.
  4. Main loop: stream x tiles [128, 240]; the vector engine multiplies
     x (stride-0 read over the e axis) against the RAW [e, 112] gates with
     the irrep 112->240 expansion fused into the access patterns
     (k-broadcast dims on the gate operand); store [128, 3000] halves on
     both HWDGE rings (sync + scalar). HBM-write-bound: ~49 MB per core at
     the observed ~390 GB/s store rate.

Measured (core 0, all 8 cores active): ~158 us vs ~127 us pure-write
roofline; DVE busy ~107 us hides under the stores.
"""

import sys
from contextlib import ExitStack

import numpy as np

try:
    import concourse.bass as bass  # noqa: F401
except ImportError:  # pragma: no cover
    sys.path.insert(0, "/opt/trn_rl_repo")
    import concourse.bass as bass

import concourse.bacc as bacc
import concourse.tile as tile
from concourse import mybir
from concourse.bass_utils import run_bass_kernel_spmd

FP32 = mybir.dt.float32

M, D = 4096, 240
NE, E_DIM, HIDDEN, NCOP = 100, 64, 256, 112
N_CORES = 8
MC = M // N_CORES          # 512 rows per core
MT = MC // 128             # 4 m-tiles of 128 rows
EC = 25                    # e-chunk size
NEC = NE // EC             # 4 e-chunks
CHUNK = EC * D             # 6000 out elements per chunk per partition
RCHUNK = EC * NCOP         # 2800 raw gate elements per chunk

# packed param layout (columns of the [128, NPARAM] tensor)
C_W2A, C_W2B = 0, 256
C_W3A, C_W3B = 512, 624
C_B1, C_B2 = 736, 738
C_W1 = 740                 # [64, 128] x 2 stacked on partition halves
C_ET = 868                 # e_featT [64, 100] duplicated on both halves
C_B3 = 968                 # [1, 112] on partition 0
NPARAM = 1080

_CACHE = {}


def _build_program():
    nc = bacc.Bacc(None, target_bir_lowering=False, debug=False)

    x_d = nc.dram_tensor("x", [MC, D], FP32, kind="ExternalInput")
    p_d = nc.dram_tensor("params", [128, NPARAM], FP32, kind="ExternalInput")
    out_d = nc.dram_tensor("out", [MC, NE * D], FP32, kind="ExternalOutput")

    with tile.TileContext(nc) as tc, ExitStack() as ctx:
        const_pool = ctx.enter_context(tc.tile_pool(name="const", bufs=1))
        mlp_pool = ctx.enter_context(tc.tile_pool(name="mlp", bufs=1))
        psum_mlp = ctx.enter_context(
            tc.tile_pool(name="psum_mlp", bufs=2, space="PSUM")
        )
        raw_pool = ctx.enter_context(tc.tile_pool(name="raw", bufs=4))
        x_pool = ctx.enter_context(tc.tile_pool(name="xin", bufs=3))
        out_pool = ctx.enter_context(tc.tile_pool(name="out", bufs=4))

        p_t = const_pool.tile([128, NPARAM], FP32)
        # critical first-layer params (W1, eT, biases) land first
        nc.sync.dma_start(out=p_t[:, C_B1:NPARAM], in_=p_d[:, C_B1:NPARAM])
        nc.scalar.dma_start(out=p_t[:, 0:C_B1], in_=p_d[:, 0:C_B1])
        ones_t = const_pool.tile([1, NE], FP32)
        nc.vector.memset(ones_t[:], 1.0)

        relu = mybir.ActivationFunctionType.Relu

        # ---- MLP: h1T = relu(W1^T e_featT + b1), two [128, 100] tiles ----
        h1T = []
        for c in range(2):
            pl, ph = 64 * c, 64 * (c + 1)
            ps = psum_mlp.tile([128, NE], FP32)
            nc.tensor.matmul(
                ps[:], p_t[pl:ph, C_W1 : C_W1 + 128], p_t[pl:ph, C_ET : C_ET + NE],
                start=True, stop=True,
            )
            h = mlp_pool.tile([128, NE], FP32, tag=f"h1T{c}")
            nc.scalar.activation(h[:], ps[:], relu, bias=p_t[:, C_B1 + c : C_B1 + c + 1])
            h1T.append(h)

        # ---- h2T = relu(W2^T h1T + b2) ----
        h2T = []
        for c in range(2):
            ps = psum_mlp.tile([128, NE], FP32)
            nc.tensor.matmul(
                ps[:], p_t[:, C_W2A + c * 128 : C_W2A + (c + 1) * 128], h1T[0][:],
                start=True, stop=False,
            )
            nc.tensor.matmul(
                ps[:], p_t[:, C_W2B + c * 128 : C_W2B + (c + 1) * 128], h1T[1][:],
                start=False, stop=True,
            )
            h = mlp_pool.tile([128, NE], FP32, tag=f"h2T{c}")
            nc.scalar.activation(h[:], ps[:], relu, bias=p_t[:, C_B2 + c : C_B2 + c + 1])
            h2T.append(h)

        # ---- gates = h2 @ W3 + b3 : psum [100, 112], partition = e ----
        psg = psum_mlp.tile([NE, NCOP], FP32)
        nc.tensor.matmul(
            psg[:], h2T[0][:], p_t[:, C_W3A : C_W3A + NCOP], start=True, stop=False
        )
        nc.tensor.matmul(
            psg[:], h2T[1][:], p_t[:, C_W3B : C_W3B + NCOP], start=False, stop=False
        )
        # += ones[100,1] @ b3[1,112]: bias along the free dim via PE
        nc.tensor.matmul(
            psg[:], ones_t[:], p_t[0:1, C_B3 : C_B3 + NCOP], start=False, stop=True
        )
        gates_t = mlp_pool.tile([NE, NCOP], FP32)
        nc.scalar.copy(gates_t[:], psg[:])

        # ---- broadcast gates to all 128 partitions ----
        # Chunk 0 takes the low-latency path: flatten onto partition 0 and
        # GPSIMD partition_broadcast (finishes before the first multiply, so
        # no SBUF-port contention with the vector engine). Chunks 1-3 go via
        # a DRAM bounce with stride-0 source reads on the then-idle DMA
        # engines; they complete long before their consumers.
        raws = []
        raw0 = raw_pool.tile([128, RCHUNK], FP32)
        st = mlp_pool.tile([1, RCHUNK], FP32)
        nc.sync.dma_start(out=st[:], in_=gates_t[0:EC, :])
        pb_i = nc.gpsimd.partition_broadcast(raw0[:], st[0:1, :])
        raws.append(raw0)

        g_dram = nc.dram_tensor("gates_scratch", [NE * NCOP], FP32)
        bounce_i = nc.sync.dma_start(
            out=g_dram[RCHUNK:], in_=gates_t[EC:NE, :]
        )
        # keep the 16 SDMA engines free for the chunk-0 stage+broadcast:
        # the bulk bounce/broadcast reads have slack until ~mid-loop
        tile.add_dep_helper(
            bounce_i.ins, pb_i.ins, sync=True,
            reason="defer bulk gate bcast behind chunk-0 broadcast",
        )
        for ec in range(1, NEC):
            raw = raw_pool.tile([128, RCHUNK], FP32)
            src = (
                g_dram[ec * RCHUNK : (ec + 1) * RCHUNK]
                .unsqueeze(0)
                .to_broadcast((128, RCHUNK))
            )
            eng = nc.sync if ec % 2 == 0 else nc.scalar
            eng.dma_start(out=raw[:], in_=src)
            raws.append(raw)

        # ---- main loop: out[m, e, d] = x[m, d] * gates[e, c(d)] ----
        # The irrep expansion (112 channels -> 240 dims) is fused into the
        # multiply via broadcast access patterns on the gate operand.
        half = CHUNK // 2
        for mt in range(MT):
            x_t = x_pool.tile([128, D], FP32)
            nc.gpsimd.dma_start(out=x_t[:], in_=x_d[mt * 128 : (mt + 1) * 128, :])
            x_v = x_t[:].unsqueeze(1).to_broadcast((128, EC, D))
            for ec in range(NEC):
                g_v = raws[ec][:].rearrange("p (e c) -> p e c", c=NCOP)
                o_t = out_pool.tile([128, CHUNK], FP32)
                o_v = o_t[:].rearrange("p (e d) -> p e d", d=D)
                nc.vector.tensor_mul(
                    o_v[:, :, 0:64], x_v[:, :, 0:64], g_v[:, :, 0:64]
                )
                nc.vector.tensor_mul(
                    o_v[:, :, 64:160].rearrange("p e (i k) -> p e i k", k=3),
                    x_v[:, :, 64:160].rearrange("p e (i k) -> p e i k", k=3),
                    g_v[:, :, 64:96].unsqueeze(3).to_broadcast((128, EC, 32, 3)),
                )
                nc.vector.tensor_mul(
                    o_v[:, :, 160:240].rearrange("p e (i k) -> p e i k", k=5),
                    x_v[:, :, 160:240].rearrange("p e (i k) -> p e i k", k=5),
                    g_v[:, :, 96:112].unsqueeze(3).to_broadcast((128, EC, 16, 5)),
                )
                # split the store across both HWDGE rings (SP + ACT)
                base = ec * CHUNK
                nc.sync.dma_start(
                    out=out_d[mt * 128 : (mt + 1) * 128, base : base + half],
                    in_=o_t[:, 0:half],
                )
                nc.scalar.dma_start(
                    out=out_d[mt * 128 : (mt + 1) * 128, base + half : base + CHUNK],
                    in_=o_t[:, half:CHUNK],
                )

    nc.compile()
    return nc


def _marshal(inputs):
    f32 = lambda a: np.ascontiguousarray(np.asarray(a, dtype=np.float32))
    x = f32(inputs["x"])
    W1, W2, W3 = f32(inputs["W1"]), f32(inputs["W2"]), f32(inputs["W3"])
    b1, b2, b3 = f32(inputs["b1"]), f32(inputs["b2"]), f32(inputs["b3"])
    eT = f32(np.asarray(inputs["e_feat"]).T)

    p = np.zeros((128, NPARAM), np.float32)
    p[:, C_W2A : C_W2A + 256] = W2[0:128]
    p[:, C_W2B : C_W2B + 256] = W2[128:256]
    p[:, C_W3A : C_W3A + NCOP] = W3[0:128]
    p[:, C_W3B : C_W3B + NCOP] = W3[128:256]
    p[:, C_B1] = b1[0:128]
    p[:, C_B1 + 1] = b1[128:256]
    p[:, C_B2] = b2[0:128]
    p[:, C_B2 + 1] = b2[128:256]
    p[0:64, C_W1 : C_W1 + 128] = W1[:, 0:128]
    p[64:128, C_W1 : C_W1 + 128] = W1[:, 128:256]
    p[0:64, C_ET : C_ET + NE] = eT
    p[64:128, C_ET : C_ET + NE] = eT
    p[0, C_B3 : C_B3 + NCOP] = b3

    return [
        {"x": x[i * MC : (i + 1) * MC], "params": p} for i in range(N_CORES)
    ]


def get_program():
    if "nc" not in _CACHE:
        _CACHE["nc"] = _build_program()
    return _CACHE["nc"]


def run(inputs, trace=False, **kwargs):
    """Run on 8 cores; returns (out [M, NE, D], BassKernelResults)."""
    nc = get_program()
    in_maps = _marshal(inputs)
    res = run_bass_kernel_spmd(
        nc, in_maps, core_ids=list(range(N_CORES)), trace=trace, **kwargs
    )
    out = np.concatenate(
        [np.asarray(res.results[i]["out"]).reshape(MC, NE, D) for i in range(N_CORES)],
        axis=0,
    )
    return out, res


def kernel(**inputs) -> np.ndarray:
    out, _ = run(inputs)
    return out



# revision 3
# speedup vs baseline: 1.2407x; 1.2407x over previous
"""Trainium2 Bass kernel for EnergyIrrepModulation (fp16 pipeline).

Computes out[m, e, d] = x[m, d] * gates_full[e, d] where
gates = MLP(e_feat) : [nE, n_copies], expanded to [nE, D] via the static
irrep index map for IRREPS = [(64, 1), (32, 3), (16, 5)].

Sharding: data-parallel over M (4096 rows -> 512 rows per core, 8 cores).
Gates/MLP params are replicated; each core redundantly computes the tiny MLP.

The kernel is HBM-write-bound (full-size output), so everything runs in
fp16 (harness tolerance 2e-2; fp16 keeps max rel err ~1e-3):
  - output stores are fp16: 24.6 MB/core instead of 49.2 MB
  - DVE tensor_tensor multiplies hit the 2x perf mode (16-bit, step-1 APs)
  - PE matmuls are single-pass (no fp32 LOW/HIGH double pumping)

Per-core device plan:
  1. All MLP params arrive packed in ONE [128, 1080] fp16 tensor; the host
     pre-transposes e_feat so no on-device transposes are needed.
  2. Tiny MLP on the tensor engine (fp16 in, f32 PSUM); biases+ReLU fused
     on the scalar engine (b3 added along the free dim with a
     ones[100,1] @ b3[1,112] matmul).
  3. Gates are expanded 112 -> 240 on the e-partitions (3 small DVE
     broadcast-AP copies), flattened to partition 0 via SBUF->SBUF DMA,
     then partition_broadcast (GPSIMD, off the DMA fabric) to all 128
     partitions in uneven e-chunks so the first multiply starts early.
  4. Main loop: one fp16 tensor_mul per (e-chunk, m-tile) with the x
     operand stride-0-broadcast over e (clean step-1 innermost on all
     APs -> 2x DVE mode); whole-tile stores alternate between the two
     HWDGE rings (sync + scalar).
"""

import sys
from contextlib import ExitStack

import numpy as np

try:
    import concourse.bass as bass  # noqa: F401
except ImportError:  # pragma: no cover
    sys.path.insert(0, "/opt/trn_rl_repo")
    import concourse.bass as bass

import concourse.bacc as bacc
import concourse.tile as tile
from concourse import mybir
from concourse.bass_utils import run_bass_kernel_spmd

FP16 = mybir.dt.float16
FP32 = mybir.dt.float32

M, D = 4096, 240
NE, E_DIM, HIDDEN, NCOP = 100, 64, 256, 112
N_CORES = 8
MC = M // N_CORES          # 512 rows per core
MT = MC // 128             # 4 m-tiles of 128 rows

# uneven e-chunks: small first chunk -> first multiply/store starts early,
# while partition_broadcast of the later chunks hides under the stores
ECH = [16, 28, 28, 28]
EOFF = [0, 16, 44, 72]

# packed param layout (columns of the [128, NPARAM] fp16 tensor)
C_W2A, C_W2B = 0, 256
C_W3A, C_W3B = 512, 624
C_B1, C_B2 = 736, 738
C_W1 = 740                 # [64, 128] x 2 stacked on partition halves
C_ET = 868                 # e_featT [64, 100] duplicated on both halves
C_B3 = 968                 # [1, 112] on partition 0
NPARAM = 1080

_CACHE = {}


def _build_program():
    nc = bacc.Bacc(None, target_bir_lowering=False, debug=False)

    x_d = nc.dram_tensor("x", [MC, D], FP16, kind="ExternalInput")
    p_d = nc.dram_tensor("params", [128, NPARAM], FP16, kind="ExternalInput")
    out_d = nc.dram_tensor("out", [MC, NE * D], FP16, kind="ExternalOutput")

    with tile.TileContext(nc) as tc, ExitStack() as ctx:
        const_pool = ctx.enter_context(tc.tile_pool(name="const", bufs=1))
        mlp_pool = ctx.enter_context(tc.tile_pool(name="mlp", bufs=1))
        psum_mlp = ctx.enter_context(
            tc.tile_pool(name="psum_mlp", bufs=2, space="PSUM")
        )
        g_pool = ctx.enter_context(tc.tile_pool(name="gates", bufs=4))
        x_pool = ctx.enter_context(tc.tile_pool(name="xin", bufs=1))
        out_pool = ctx.enter_context(tc.tile_pool(name="out", bufs=4))

        p_t = const_pool.tile([128, NPARAM], FP16)
        # critical first-layer params (W1, eT, biases) land first
        nc.sync.dma_start(out=p_t[:, C_B1:NPARAM], in_=p_d[:, C_B1:NPARAM])
        nc.scalar.dma_start(out=p_t[:, 0:C_B1], in_=p_d[:, 0:C_B1])
        ones_t = const_pool.tile([1, NE], FP16)
        nc.vector.memset(ones_t[:], 1.0)

        # x for the whole core, loaded once: [128, MT*D], row p holds the
        # 4 m-tile rows t*128+p  (SWDGE so the HWDGE rings stay free)
        x_t = x_pool.tile([128, MT * D], FP16)
        nc.gpsimd.dma_start(
            out=x_t[:].rearrange("p (t d) -> p t d", d=D),
            in_=x_d[:].rearrange("(t p) d -> p t d", p=128),
        )

        relu = mybir.ActivationFunctionType.Relu

        # ---- MLP: h1T = relu(W1^T e_featT + b1), two [128, 100] tiles ----
        h1T = []
        for c in range(2):
            pl, ph = 64 * c, 64 * (c + 1)
            ps = psum_mlp.tile([128, NE], FP32)
            nc.tensor.matmul(
                ps[:], p_t[pl:ph, C_W1 : C_W1 + 128], p_t[pl:ph, C_ET : C_ET + NE],
                start=True, stop=True,
            )
            h = mlp_pool.tile([128, NE], FP16, tag=f"h1T{c}")
            nc.scalar.activation(h[:], ps[:], relu, bias=p_t[:, C_B1 + c : C_B1 + c + 1])
            h1T.append(h)

        # ---- h2T = relu(W2^T h1T + b2) ----
        h2T = []
        for c in range(2):
            ps = psum_mlp.tile([128, NE], FP32)
            nc.tensor.matmul(
                ps[:], p_t[:, C_W2A + c * 128 : C_W2A + (c + 1) * 128], h1T[0][:],
                start=True, stop=False,
            )
            nc.tensor.matmul(
                ps[:], p_t[:, C_W2B + c * 128 : C_W2B + (c + 1) * 128], h1T[1][:],
                start=False, stop=True,
            )
            h = mlp_pool.tile([128, NE], FP16, tag=f"h2T{c}")
            nc.scalar.activation(h[:], ps[:], relu, bias=p_t[:, C_B2 + c : C_B2 + c + 1])
            h2T.append(h)

        # ---- gates = h2 @ W3 + b3 : psum [100, 112], partition = e ----
        psg = psum_mlp.tile([NE, NCOP], FP32)
        nc.tensor.matmul(
            psg[:], h2T[0][:], p_t[:, C_W3A : C_W3A + NCOP], start=True, stop=False
        )
        nc.tensor.matmul(
            psg[:], h2T[1][:], p_t[:, C_W3B : C_W3B + NCOP], start=False, stop=False
        )
        # += ones[100,1] @ b3[1,112]: bias along the free dim via PE
        nc.tensor.matmul(
            psg[:], ones_t[:], p_t[0:1, C_B3 : C_B3 + NCOP], start=False, stop=True
        )
        graw = mlp_pool.tile([NE, NCOP], FP16)
        nc.scalar.copy(graw[:], psg[:])

        # ---- expand 112 -> 240 on the e-partitions (fp16) ----
        gfull_e = mlp_pool.tile([NE, D], FP16)
        nc.vector.tensor_copy(gfull_e[:, 0:64], graw[:, 0:64])
        nc.vector.tensor_copy(
            gfull_e[:, 64:160].rearrange("e (i k) -> e i k", k=3),
            graw[:, 64:96].unsqueeze(2).to_broadcast((NE, 32, 3)),
        )
        nc.vector.tensor_copy(
            gfull_e[:, 160:240].rearrange("e (i k) -> e i k", k=5),
            graw[:, 96:112].unsqueeze(2).to_broadcast((NE, 16, 5)),
        )

        # ---- flatten to partition 0, then broadcast chunks to 128 parts ----
        flat = mlp_pool.tile([1, NE * D], FP16)
        nc.sync.dma_start(out=flat[:], in_=gfull_e[:])

        gchunks = []
        for ci, (sz, off) in enumerate(zip(ECH, EOFF)):
            g = g_pool.tile([128, sz * D], FP16, tag="g")
            nc.gpsimd.partition_broadcast(
                g[:], flat[0:1, off * D : (off + sz) * D]
            )
            gchunks.append(g)

        # ---- main loop: out[m, e, d] = x[m, d] * gates_full[e, d] ----
        si = 0
        for ci, (sz, off) in enumerate(zip(ECH, EOFF)):
            g_v = gchunks[ci][:].rearrange("p (e d) -> p e d", d=D)
            for mt in range(MT):
                x_v = (
                    x_t[:, mt * D : (mt + 1) * D]
                    .unsqueeze(1)
                    .to_broadcast((128, sz, D))
                )
                o_t = out_pool.tile([128, sz * D], FP16, tag="o")
                o_v = o_t[:].rearrange("p (e d) -> p e d", d=D)
                nc.vector.tensor_mul(o_v, x_v, g_v)
                eng = nc.sync if si % 2 == 0 else nc.scalar
                eng.dma_start(
                    out=out_d[
                        mt * 128 : (mt + 1) * 128, off * D : (off + sz) * D
                    ],
                    in_=o_t[:],
                )
                si += 1

    nc.compile()
    return nc


def _marshal(inputs):
    f16 = lambda a: np.ascontiguousarray(np.asarray(a, dtype=np.float16))
    x = f16(inputs["x"])
    W1, W2, W3 = f16(inputs["W1"]), f16(inputs["W2"]), f16(inputs["W3"])
    b1, b2, b3 = f16(inputs["b1"]), f16(inputs["b2"]), f16(inputs["b3"])
    eT = f16(np.asarray(inputs["e_feat"]).T)

    p = np.zeros((128, NPARAM), np.float16)
    p[:, C_W2A : C_W2A + 256] = W2[0:128]
    p[:, C_W2B : C_W2B + 256] = W2[128:256]
    p[:, C_W3A : C_W3A + NCOP] = W3[0:128]
    p[:, C_W3B : C_W3B + NCOP] = W3[128:256]
    p[:, C_B1] = b1[0:128]
    p[:, C_B1 + 1] = b1[128:256]
    p[:, C_B2] = b2[0:128]
    p[:, C_B2 + 1] = b2[128:256]
    p[0:64, C_W1 : C_W1 + 128] = W1[:, 0:128]
    p[64:128, C_W1 : C_W1 + 128] = W1[:, 128:256]
    p[0:64, C_ET : C_ET + NE] = eT
    p[64:128, C_ET : C_ET + NE] = eT
    p[0, C_B3 : C_B3 + NCOP] = b3

    return [
        {"x": x[i * MC : (i + 1) * MC], "params": p} for i in range(N_CORES)
    ]


def get_program():
    if "nc" not in _CACHE:
        _CACHE["nc"] = _build_program()
    return _CACHE["nc"]


def run(inputs, trace=False, **kwargs):
    """Run on 8 cores; returns (out [M, NE, D], BassKernelResults)."""
    nc = get_program()
    in_maps = _marshal(inputs)
    res = run_bass_kernel_spmd(
        nc, in_maps, core_ids=list(range(N_CORES)), trace=trace, **kwargs
    )
    out = np.concatenate(
        [
            np.asarray(res.results[i]["out"])
            .astype(np.float32)
            .reshape(MC, NE, D)
            for i in range(N_CORES)
        ],
        axis=0,
    )
    return out, res


def kernel(**inputs) -> np.ndarray:
    out, _ = run(inputs)
    return out


# revision 7
# speedup vs baseline: 1.8223x; 1.4687x over previous
"""Trainium2 Bass kernel for EnergyIrrepModulation (fp16 pipeline).

Computes out[m, e, d] = x[m, d] * gates_full[e, d] where
gates = MLP(e_feat) : [nE, n_copies], expanded to [nE, D] via the static
irrep index map for IRREPS = [(64, 1), (32, 3), (16, 5)].

Sharding: data-parallel over M (4096 rows -> 512 rows per core, 8 cores).
Gates/MLP params are replicated; each core redundantly computes the tiny MLP.

The kernel is HBM-write-bound (full-size output), so everything runs in
fp16 (harness tolerance 2e-2; fp16 keeps max rel err ~1e-3):
  - output stores are fp16: 24.6 MB/core instead of 49.2 MB
  - DVE tensor_tensor multiplies hit the 2x perf mode (16-bit, step-1 APs)
  - PE matmuls are single-pass (no fp32 LOW/HIGH double pumping)

Per-core device plan:
  1. All MLP params arrive packed in ONE [128, 1080] fp16 tensor; the host
     pre-transposes e_feat so no on-device transposes are needed.
  2. Tiny MLP on the tensor engine (fp16 in, f32 PSUM); biases+ReLU fused
     on the scalar engine (b3 added along the free dim with a
     ones[100,1] @ b3[1,112] matmul).
  3. Gates are expanded 112 -> 240 on the e-partitions (3 small DVE
     broadcast-AP copies), flattened to partition 0 via SBUF->SBUF DMA,
     then broadcast to all 128 partitions via PE ones[1,128]-matmuls into
     PSUM (512-col bank pieces) copied out by ACT (chunk 0 by DVE, which
     is idle pre-loop).  partition_broadcast is deliberately NOT used: its
     Q7 SBUF traffic nearly stalls concurrent DVE work (measured 3.5us ->
     12.5us on an overlapped tensor_mul).
  4. Main loop: one fp16 tensor_mul per (e-chunk, m-tile) with the x
     operand stride-0-broadcast over e (2x DVE mode confirmed with the
     broadcast AP); whole-tile stores alternate between the two HWDGE
     rings (sync + scalar).
"""

import sys
from contextlib import ExitStack

import numpy as np

try:
    import concourse.bass as bass  # noqa: F401
except ImportError:  # pragma: no cover
    sys.path.insert(0, "/opt/trn_rl_repo")
    import concourse.bass as bass

import concourse.bacc as bacc
import concourse.tile as tile
from concourse import mybir
from concourse.bass_utils import run_bass_kernel_spmd

FP16 = mybir.dt.float16
FP32 = mybir.dt.float32

M, D = 4096, 240
NE, E_DIM, HIDDEN, NCOP = 100, 64, 256, 112
N_CORES = 8
MC = M // N_CORES          # 512 rows per core
MT = MC // 128             # 4 m-tiles of 128 rows

# uneven e-chunks: small first chunk -> first multiply/store starts early,
# while partition_broadcast of the later chunks hides under the stores
ECH = [16, 28, 28, 28]
EOFF = [0, 16, 44, 72]

# packed param layout (columns of the [128, NPARAM] fp16 tensor)
C_W2A, C_W2B = 0, 256
C_W3A, C_W3B = 512, 624
C_B1, C_B2 = 736, 738
C_W1 = 740                 # [64, 128] x 2 stacked on partition halves
C_ET = 868                 # e_featT [64, 100] duplicated on both halves
C_B3 = 968                 # [1, 112] on partition 0
NPARAM = 1080

_CACHE = {}


def _build_program():
    nc = bacc.Bacc(None, target_bir_lowering=False, debug=False)

    x_d = nc.dram_tensor("x", [MC, D], FP16, kind="ExternalInput")
    p_d = nc.dram_tensor("params", [128, NPARAM], FP16, kind="ExternalInput")
    out_d = nc.dram_tensor("out", [MC, NE * D], FP16, kind="ExternalOutput")

    with tile.TileContext(nc) as tc, ExitStack() as ctx:
        const_pool = ctx.enter_context(tc.tile_pool(name="const", bufs=1))
        mlp_pool = ctx.enter_context(tc.tile_pool(name="mlp", bufs=1))
        psum_mlp = ctx.enter_context(
            tc.tile_pool(name="psum_mlp", bufs=2, space="PSUM")
        )
        psum_bc = ctx.enter_context(
            tc.tile_pool(name="psum_bc", bufs=4, space="PSUM")
        )
        g_pool = ctx.enter_context(tc.tile_pool(name="gates", bufs=4))
        x_pool = ctx.enter_context(tc.tile_pool(name="xin", bufs=1))
        out_pool = ctx.enter_context(tc.tile_pool(name="out", bufs=4))

        p_t = const_pool.tile([128, NPARAM], FP16)
        # critical first-layer params (W1, eT, biases) land first
        nc.sync.dma_start(out=p_t[:, C_B1:NPARAM], in_=p_d[:, C_B1:NPARAM])
        nc.scalar.dma_start(out=p_t[:, 0:C_B1], in_=p_d[:, 0:C_B1])
        ones_t = const_pool.tile([1, 128], FP16)
        nc.vector.memset(ones_t[:], 1.0)

        # x for the whole core, loaded once: [128, MT*D], row p holds the
        # 4 m-tile rows t*128+p  (SWDGE so the HWDGE rings stay free)
        x_t = x_pool.tile([128, MT * D], FP16)
        nc.gpsimd.dma_start(
            out=x_t[:].rearrange("p (t d) -> p t d", d=D),
            in_=x_d[:].rearrange("(t p) d -> p t d", p=128),
        )

        relu = mybir.ActivationFunctionType.Relu

        # ---- MLP: h1T = relu(W1^T e_featT + b1), two [128, 100] tiles ----
        h1T = []
        for c in range(2):
            pl, ph = 64 * c, 64 * (c + 1)
            ps = psum_mlp.tile([128, NE], FP32)
            nc.tensor.matmul(
                ps[:], p_t[pl:ph, C_W1 : C_W1 + 128], p_t[pl:ph, C_ET : C_ET + NE],
                start=True, stop=True,
            )
            h = mlp_pool.tile([128, NE], FP16, tag=f"h1T{c}")
            nc.scalar.activation(h[:], ps[:], relu, bias=p_t[:, C_B1 + c : C_B1 + c + 1])
            h1T.append(h)

        # ---- h2T = relu(W2^T h1T + b2) ----
        h2T = []
        for c in range(2):
            ps = psum_mlp.tile([128, NE], FP32)
            nc.tensor.matmul(
                ps[:], p_t[:, C_W2A + c * 128 : C_W2A + (c + 1) * 128], h1T[0][:],
                start=True, stop=False,
            )
            nc.tensor.matmul(
                ps[:], p_t[:, C_W2B + c * 128 : C_W2B + (c + 1) * 128], h1T[1][:],
                start=False, stop=True,
            )
            h = mlp_pool.tile([128, NE], FP16, tag=f"h2T{c}")
            nc.scalar.activation(h[:], ps[:], relu, bias=p_t[:, C_B2 + c : C_B2 + c + 1])
            h2T.append(h)

        # ---- gates = h2 @ W3 + b3 : psum [100, 112], partition = e ----
        psg = psum_mlp.tile([NE, NCOP], FP32)
        nc.tensor.matmul(
            psg[:], h2T[0][:], p_t[:, C_W3A : C_W3A + NCOP], start=True, stop=False
        )
        nc.tensor.matmul(
            psg[:], h2T[1][:], p_t[:, C_W3B : C_W3B + NCOP], start=False, stop=False
        )
        # += ones[100,1] @ b3[1,112]: bias along the free dim via PE
        nc.tensor.matmul(
            psg[:], ones_t[0:1, 0:NE], p_t[0:1, C_B3 : C_B3 + NCOP],
            start=False, stop=True,
        )
        graw = mlp_pool.tile([NE, NCOP], FP16)
        nc.scalar.copy(graw[:], psg[:])

        # ---- expand 112 -> 240 on the e-partitions (fp16) ----
        gfull_e = mlp_pool.tile([NE, D], FP16)
        nc.vector.tensor_copy(gfull_e[:, 0:64], graw[:, 0:64])
        nc.vector.tensor_copy(
            gfull_e[:, 64:160].rearrange("e (i k) -> e i k", k=3),
            graw[:, 64:96].unsqueeze(2).to_broadcast((NE, 32, 3)),
        )
        nc.vector.tensor_copy(
            gfull_e[:, 160:240].rearrange("e (i k) -> e i k", k=5),
            graw[:, 96:112].unsqueeze(2).to_broadcast((NE, 16, 5)),
        )

        # ---- flatten to partition 0, then broadcast chunks to 128 parts ----
        flat = mlp_pool.tile([1, NE * D], FP16)
        nc.sync.dma_start(out=flat[:], in_=gfull_e[:])

        # PE ones-matmul broadcast: psum piece [128, <=512] = ones^T @ flat
        # slice, copied to the chunk tile (DVE for chunk 0 — idle pre-loop;
        # ACT for the rest so DVE stays on the multiplies).
        gchunks = []
        for ci, (sz, off) in enumerate(zip(ECH, EOFF)):
            g = g_pool.tile([128, sz * D], FP16, tag="g")
            n = sz * D
            pos = 0
            while pos < n:
                w = min(512, n - pos)
                ps = psum_bc.tile([128, 512], FP32, tag="bcp")
                nc.tensor.matmul(
                    ps[:, 0:w],
                    ones_t[:],
                    flat[0:1, off * D + pos : off * D + pos + w],
                    start=True, stop=True,
                )
                if ci == 0:
                    nc.vector.tensor_copy(g[:, pos : pos + w], ps[:, 0:w])
                else:
                    nc.scalar.copy(g[:, pos : pos + w], ps[:, 0:w])
                pos += w
            gchunks.append(g)

        # ---- main loop: out[m, e, d] = x[m, d] * gates_full[e, d] ----
        si = 0
        for ci, (sz, off) in enumerate(zip(ECH, EOFF)):
            g_v = gchunks[ci][:].rearrange("p (e d) -> p e d", d=D)
            for mt in range(MT):
                x_v = (
                    x_t[:, mt * D : (mt + 1) * D]
                    .unsqueeze(1)
                    .to_broadcast((128, sz, D))
                )
                o_t = out_pool.tile([128, sz * D], FP16, tag="o")
                o_v = o_t[:].rearrange("p (e d) -> p e d", d=D)
                nc.vector.tensor_mul(o_v, x_v, g_v)
                eng = nc.sync if si % 2 == 0 else nc.scalar
                eng.dma_start(
                    out=out_d[
                        mt * 128 : (mt + 1) * 128, off * D : (off + sz) * D
                    ],
                    in_=o_t[:],
                )
                si += 1

    nc.compile()
    return nc


def _marshal(inputs):
    f16 = lambda a: np.ascontiguousarray(np.asarray(a, dtype=np.float16))
    x = f16(inputs["x"])
    W1, W2, W3 = f16(inputs["W1"]), f16(inputs["W2"]), f16(inputs["W3"])
    b1, b2, b3 = f16(inputs["b1"]), f16(inputs["b2"]), f16(inputs["b3"])
    eT = f16(np.asarray(inputs["e_feat"]).T)

    p = np.zeros((128, NPARAM), np.float16)
    p[:, C_W2A : C_W2A + 256] = W2[0:128]
    p[:, C_W2B : C_W2B + 256] = W2[128:256]
    p[:, C_W3A : C_W3A + NCOP] = W3[0:128]
    p[:, C_W3B : C_W3B + NCOP] = W3[128:256]
    p[:, C_B1] = b1[0:128]
    p[:, C_B1 + 1] = b1[128:256]
    p[:, C_B2] = b2[0:128]
    p[:, C_B2 + 1] = b2[128:256]
    p[0:64, C_W1 : C_W1 + 128] = W1[:, 0:128]
    p[64:128, C_W1 : C_W1 + 128] = W1[:, 128:256]
    p[0:64, C_ET : C_ET + NE] = eT
    p[64:128, C_ET : C_ET + NE] = eT
    p[0, C_B3 : C_B3 + NCOP] = b3

    return [
        {"x": x[i * MC : (i + 1) * MC], "params": p} for i in range(N_CORES)
    ]


def get_program():
    if "nc" not in _CACHE:
        _CACHE["nc"] = _build_program()
    return _CACHE["nc"]


def run(inputs, trace=False, **kwargs):
    """Run on 8 cores; returns (out [M, NE, D], BassKernelResults)."""
    nc = get_program()
    in_maps = _marshal(inputs)
    res = run_bass_kernel_spmd(
        nc, in_maps, core_ids=list(range(N_CORES)), trace=trace, **kwargs
    )
    out = np.concatenate(
        [
            np.asarray(res.results[i]["out"])
            .astype(np.float32)
            .reshape(MC, NE, D)
            for i in range(N_CORES)
        ],
        axis=0,
    )
    return out, res


def kernel(**inputs) -> np.ndarray:
    out, _ = run(inputs)
    return out
